# revision 36
# baseline (speedup 1.0000x reference)
"""Trainium2 8-core kernel for a dense pre-norm transformer block.

Reference: h=LN1(x); qkv=h@w_qkv; causal MHA (16 heads, Dh=64);
x+=o@w_out; h2=LN2(x); x+=gelu(h2@w1+b1)@w2+b2.

Sharding:
  - Attention Megatron-TP8: heads 2c,2c+1 on core c (w_qkv column-shard,
    w_out row-shard), ReduceScatter of the out-projection in 4 token
    quarters (core c owns strided pieces {q*1024 + c*128 .. +128}).
  - MLP sequence-parallel: every core holds FULL w1/w2 and runs the MLP
    only for its own 512 tokens.  No AllGather, no second
    ReduceScatter; RS1 is the only large collective.  w1 is resident,
    w2 is streamed in 2MB chunks during MLP2.
  - LN1 stats shard-local (bn_stats on own tokens) + tiny AllGather;
    LN mean-subtraction folded into the qkv matmul as a rank-1 update;
    gains/biases folded host-side.

Precision: fp8(e4m3) DoubleRow matmuls for qkv (x scaled x8, w x512),
fp8 scores + fp8 AV (q,k,v stored x16; off-diagonal k-chunk pairs run
DoubleRow, causal-diagonal chunks run single fp8 matmuls with masks).
Out-projection and both MLP matmuls stay bf16.  fp32 PSUM everywhere;
fp32 residual stream.
"""
import numpy as np

import concourse.bass as bass
import concourse.mybir as mybir
import concourse.tile as tile
from concourse import bacc
from concourse import bass_utils
from concourse.masks import make_identity

F32 = mybir.dt.float32
BF = mybir.dt.bfloat16
E4 = mybir.dt.float8e4
AF = mybir.ActivationFunctionType
DR = mybir.MatmulPerfMode.DoubleRow

NCORES = 8
B, L, D = 2, 2048, 1024
T = B * L              # 4096 tokens
TSH = T // NCORES      # 512 tokens per core (4 pieces of 128)
DH = 64                # head dim
HL = 2                 # heads per core
DLOC = HL * DH         # 128 local head features
LN_EPS = 1e-5
NT = T // 512          # 8 token tiles of 512
ND = D // 128          # 8 feature chunks
MH = 4096              # full MLP hidden
NM = MH // 128         # 32 hidden chunks

S_X = 8.0              # fp8 scale on x
S_W = 512.0            # fp8 scale on w_qkv
QS = 16.0              # q,k,v storage scale
# exp(scale * pst): pst = (16q).(16k), ref scale 1/8 -> 1/(8*256)
EXP_SCALE = 0.125 / (QS * QS)
# u-term: praw_psum = (S_X*S_W) * x@w ; want 16*praw*rstd
K16S = QS / (S_X * S_W)   # 2^-8, exact in bf16

_CACHE = {}


def build():
    if "nc" in _CACHE:
        return _CACHE["nc"]
    nc = bacc.Bacc("TRN2", target_bir_lowering=False, debug=False,
                   num_devices=NCORES)

    xt_in = nc.dram_tensor("xt", [ND, 128, T], E4, kind="ExternalInput")
    xsh_in = nc.dram_tensor("xsh", [TSH, D], BF, kind="ExternalInput")
    xbf_in = nc.dram_tensor("xbf", [T, D], BF, kind="ExternalInput")
    wqkv_in = nc.dram_tensor("wqkv", [128, 4, 2, 3 * DLOC], E4,
                             kind="ExternalInput")
    nws_in = nc.dram_tensor("nws", [128, 3], F32, kind="ExternalInput")
    wout_in = nc.dram_tensor("wout", [DLOC, D], BF, kind="ExternalInput")
    w1_in = nc.dram_tensor("w1", [128, NM, ND, 128], BF,
                           kind="ExternalInput")
    b1g_in = nc.dram_tensor("b1g", [128, NM], F32, kind="ExternalInput")
    w2_in = nc.dram_tensor("w2", [128, NM, D], BF, kind="ExternalInput")
    masks_in = nc.dram_tensor("masks", [4, 128, 512], E4,
                              kind="ExternalInput")
    out_ext = nc.dram_tensor("out", [TSH, D], F32, kind="ExternalOutput")

    rg = [list(range(NCORES))]

    with tile.TileContext(nc) as tc:
        with (
            tc.tile_pool(name="const", bufs=1) as const,
            tc.tile_pool(name="dram", bufs=1, space="DRAM") as dram,
        ):
            # ---- DRAM scratch for collectives ----
            st_ag_in = dram.tile([8, 128], BF)       # (piece q, mean/rstd)
            st_ag_out = dram.tile([64, 128], BF, addr_space="Shared")
            rs1_in = [dram.tile([1024, D], BF, name=f"rs1_in{q}")
                      for q in range(4)]
            rs1_out = [dram.tile([128, D], BF, name=f"rs1_out{q}")
                       for q in range(4)]
            warm_in = dram.tile([8, 16], BF)
            warm_out = dram.tile([64, 16], BF, addr_space="Shared")

            # ---- constants ----
            ident = const.tile([128, 128], F32)
            make_identity(nc, ident[:])
            ident_bf = const.tile([128, 128], BF)
            make_identity(nc, ident_bf[:])
            ones_row = const.tile([1, 128], BF)
            nc.vector.memset(ones_row[:], 1.0)
            k16s_row = const.tile([1, 128], BF)
            nc.vector.memset(k16s_row[:], K16S)
            neg16_row = const.tile([1, 128], BF)
            nc.vector.memset(neg16_row[:], -QS)
            eps128 = const.tile([128, 1], F32)
            nc.vector.memset(eps128[:], LN_EPS)

            # residual stream + h2^T live to the very end
            resid_pool_cm = tc.tile_pool(name="resid", bufs=1)
            resid_pool = resid_pool_cm.__enter__()
            x2_sb = resid_pool.tile([128, 4, D], BF)
            h2T = resid_pool.tile([128, ND, TSH], BF)
            g1h = [resid_pool.tile([128, NM, 256], BF, name=f"g1_{h}")
                   for h in range(2)]


            # x shard, preloaded early (residual adds in s4)
            xsv_pool_cm = tc.tile_pool(name="xsv", bufs=1)
            xsv_pool = xsv_pool_cm.__enter__()
            xsv = xsv_pool.tile([128, 4, D], BF)

            # attention activations + small weights: die after oproj(3)
            attn_pool_cm = tc.tile_pool(name="attnp", bufs=1)
            attn_pool = attn_pool_cm.__enter__()
            qT = attn_pool.tile([128, T], E4)     # 16*q
            kT = attn_pool.tile([128, T], E4)     # 16*k
            vT = attn_pool.tile([128, T], BF)     # 16*v
            oT = attn_pool.tile([128, T], BF)     # o (exact)
            masks_sb = [attn_pool.tile([128, 512], E4, name=f"mask{m}")
                        for m in range(4)]
            wout_sb = attn_pool.tile([DLOC, D], BF)
            nws_sb = attn_pool.tile([128, 3], F32)
            for m in range(4):
                nc.scalar.dma_start(masks_sb[m][:], masks_in.ap()[m])
            nc.scalar.dma_start(nws_sb[:], nws_in.ap())

            # ========== stage 0: warm collective + local LN1 stats ======
            # per-token LN1 stats, one row each (partition base 0)
            mean_sb = attn_pool.tile([1, T // 128, 128], BF)
            rstd_sb = attn_pool.tile([1, T // 128, 128], BF)

            s0_cm = tc.tile_pool(name="s0", bufs=4)
            s0 = s0_cm.__enter__()
            ps0_cm = tc.tile_pool(name="ps0", bufs=1, space="PSUM")
            ps0 = ps0_cm.__enter__()
            ps_st_cm = tc.tile_pool(name="ps_st", bufs=2, space="PSUM")
            ps_st = ps_st_cm.__enter__()
            # absorb first-collective init latency with a no-dep dummy
            wtile = s0.tile([8, 16], BF, tag="wtile", name="wtile")
            nc.vector.memset(wtile[:], 0.0)
            nc.scalar.dma_start(warm_in[:], wtile[:])
            nc.gpsimd.collective_compute(
                "AllGather", mybir.AluOpType.bypass, replica_groups=rg,
                ins=[warm_in[:].opt()], outs=[warm_out[:].opt()])


            def do_stats(tc_):
                """LN1 stats for token chunk tc_ (128 tokens), all local."""
                eng = nc.vector
                xb = s0.tile([128, D], BF, tag="xb", name="xb")
                nc.scalar.dma_start(
                    xb[:], xbf_in.ap()[tc_ * 128:(tc_ + 1) * 128, :])
                stats = s0.tile([128, 2, 6], F32, tag="stats", name="stats")
                xv = xb[:].rearrange("p (s f) -> p s f", s=2)
                for s in range(2):
                    eng.bn_stats(stats[:, s, :], xv[:, s, :])
                mv = s0.tile([128, 2], F32, tag="mv", name="mv")
                eng.bn_aggr(mv[:], stats[:])
                rstd0 = s0.tile([128, 1], F32, tag="rstd0", name="rstd0")
                nc.scalar.activation(rstd0[:], mv[:, 1:2], AF.Sqrt,
                                     bias=eps128[:])
                nc.vector.reciprocal_approx_fast(rstd0[:], rstd0[:])
                stp_m = ps0.tile([1, 128], F32, tag="stp", name="stp_m")
                nc.tensor.transpose(stp_m[:], mv[:, 0:1], ident[:])
                nc.scalar.copy(mean_sb[:, tc_, :], stp_m[:])
                stp_r = ps0.tile([1, 128], F32, tag="stp", name="stp_r")
                nc.tensor.transpose(stp_r[:], rstd0[:], ident[:])
                nc.scalar.copy(rstd_sb[:, tc_, :], stp_r[:])

            # ================= stage 1: LN1 + qkv (fp8 DoubleRow) ========
            s1_tmp_cm = tc.tile_pool(name="s1_tmp", bufs=3)
            s1_tmp = s1_tmp_cm.__enter__()
            s1_stat_cm = tc.tile_pool(name="s1_stat", bufs=1)
            s1_stat = s1_stat_cm.__enter__()
            praw_pool_cm = tc.tile_pool(name="s1_praw", bufs=12)
            praw_pool = praw_pool_cm.__enter__()
            s1_x_cm = tc.tile_pool(name="s1_x", bufs=1)
            s1_x = s1_x_cm.__enter__()
            ps_qkv_cm = tc.tile_pool(name="ps_qkv", bufs=3, space="PSUM")
            ps_qkv = ps_qkv_cm.__enter__()
            praws = {}

            # whole x^T resident in fp8: [128, d-chunk, tok]
            wqkv_sb = s1_x.tile([128, 4, 2, 3 * DLOC], E4)
            nc.sync.dma_start(wqkv_sb[:], wqkv_in.ap())
            xts_p = [s1_x.tile([128, 2, T], E4, name=f"xts{p}")
                     for p in range(4)]
            for d in range(ND):
                nc.sync.dma_start(xts_p[d // 2][:, d % 2, :],
                                  xt_in.ap()[d])
            for q in range(4):
                nc.sync.dma_start(
                    xsv[:, q, :], xsh_in.ap()[q * 128:(q + 1) * 128, :])
            nc.sync.dma_start(wout_sb[:], wout_in.ap())

            def do_s1_mm(tt):
                praw3 = []
                for m in range(3):
                    ps_q = ps_qkv.tile([128, 512], F32, tag="ps_q",
                                       name="ps_q")
                    for p in range(4):
                        nc.tensor.matmul(
                            ps_q[:],
                            wqkv_sb[:, p, :, m * 128:(m + 1) * 128],
                            xts_p[p][:, :, tt * 512:(tt + 1) * 512],
                            start=(p == 0), stop=(p == 3),
                            perf_mode=DR)
                    praw = praw_pool.tile([128, 512], BF, tag="praw",
                                          name="praw")
                    nc.scalar.copy(praw[:], ps_q[:])
                    praw3.append(praw)
                praws[tt] = praw3

            def do_s1_fin(tt):
                praw3 = praws[tt]
                mean_v = mean_sb[0:1, 4 * tt:4 * tt + 4, :].rearrange(
                    "p a f -> p (a f)")
                rstd_v = rstd_sb[0:1, 4 * tt:4 * tt + 4, :].rearrange(
                    "p a f -> p (a f)")
                mr = s1_stat.tile([1, 512], BF, tag="mr", name="mr")
                nc.vector.tensor_mul(mr[:], mean_v, rstd_v)
                # rstd_b = 2^-8 * rstd, broadcast to 128 partitions
                rstd_b = ps_st.tile([128, 512], F32, tag="pst",
                                    name="rstd_b")
                nc.tensor.matmul(rstd_b[:], k16s_row[:], rstd_v,
                                 start=True, stop=True)
                # mr_b = -16 * mean * rstd, broadcast
                mr_b = ps_st.tile([128, 512], F32, tag="pst", name="mr_b")
                nc.tensor.matmul(mr_b[:], neg16_row[:], mr[:],
                                 start=True, stop=True)
                outs = (qT, kT, vT)
                for m in (2, 0, 1):
                    u = s1_tmp.tile([128, 512], BF, tag="pre", name="u")
                    nc.vector.tensor_mul(u[:], praw3[m][:], rstd_b[:])
                    nc.vector.scalar_tensor_tensor(
                        out=outs[m][:, tt * 512:(tt + 1) * 512],
                        in0=mr_b[:], scalar=nws_sb[:, m:m + 1],
                        in1=u[:], op0=mybir.AluOpType.mult,
                        op1=mybir.AluOpType.add)
                del praws[tt]

            # ---------------- stage-1 schedule ----------------
            for tc_ in range(12):
                do_stats(tc_)
            do_s1_mm(0)
            do_s1_mm(1)
            do_s1_mm(2)
            do_s1_fin(0)
            for tt in range(3, 8):
                do_s1_mm(tt)
                for tc_ in range(4 * tt, 4 * tt + 4):
                    do_stats(tc_)
                do_s1_fin(tt - 2)
            do_s1_fin(6)
            do_s1_fin(7)

            ps_qkv_cm.__exit__(None, None, None)
            ps_st_cm.__exit__(None, None, None)
            ps0_cm.__exit__(None, None, None)
            s1_x_cm.__exit__(None, None, None)   # frees xts + wqkv
            praw_pool_cm.__exit__(None, None, None)
            s1_stat_cm.__exit__(None, None, None)
            s1_tmp_cm.__exit__(None, None, None)
            s0_cm.__exit__(None, None, None)

            # MLP w1 resident; streams in under attention
            w1_pool_cm = tc.tile_pool(name="w1pool", bufs=1)
            w1_pool = w1_pool_cm.__enter__()
            b1g_pool_cm = tc.tile_pool(name="b1gpool", bufs=1)
            b1g_pool = b1g_pool_cm.__enter__()
            b1g_sb = b1g_pool.tile([128, NM], F32)
            nc.scalar.dma_start(b1g_sb[:], b1g_in.ap())
            w1_sb = w1_pool.tile([128, NM, ND, 128], BF)
            for m2 in range(8):
                nc.sync.dma_start(
                    w1_sb[:, 4 * m2:4 * (m2 + 1), :, :],
                    w1_in.ap()[:, 4 * m2:4 * (m2 + 1), :, :])

            # ============ attention pools ============
            s2_vaug_cm = tc.tile_pool(name="s2_vaug", bufs=1)
            s2_vaug = s2_vaug_cm.__enter__()
            s2_exp_cm = tc.tile_pool(name="s2_exp", bufs=2)
            s2_exp = s2_exp_cm.__enter__()
            s2_misc_cm = tc.tile_pool(name="s2_misc", bufs=2)
            s2_misc = s2_misc_cm.__enter__()
            s3_r1_cm = tc.tile_pool(name="s3_r1", bufs=3)
            s3_r1 = s3_r1_cm.__enter__()
            s4_t_cm = tc.tile_pool(name="s4_t", bufs=2)
            s4_t = s4_t_cm.__enter__()
            ps_vt_cm = tc.tile_pool(name="ps_vt", bufs=2, space="PSUM")
            ps_vt = ps_vt_cm.__enter__()
            ps_sc_cm = tc.tile_pool(name="ps_sc", bufs=3, space="PSUM")
            ps_sc = ps_sc_cm.__enter__()
            ps_o_cm = tc.tile_pool(name="ps_o", bufs=1, space="PSUM")
            ps_o = ps_o_cm.__enter__()

            vaug_cur = [None]

            def do_attn(b, js):
                tok0 = b * L
                if js[0] == 0:
                    # slot padded to 96: dual-fp8 ldweights needs the
                    # pair stride to be a multiple of 32
                    vaug = s2_vaug.tile([128, HL, L // 128, 96], E4,
                                        tag="vaug", name="vaug")
                    nc.vector.memset(vaug[:, :, :, DH:96], 0.0)
                    # denominator column: scale cancels against 16*v
                    nc.vector.memset(vaug[:, :, :, DH:DH + 1], QS)
                    vaug_cur[0] = vaug
                vaug = vaug_cur[0]
                kc_range = range(0, 8) if js[0] == 0 else range(8, 16)
                for hl in range(HL):
                    hrow = hl * DH
                    vT_u = vT[hrow:hrow + DH, tok0:tok0 + L]
                    for kc in kc_range:
                        pv = ps_vt.tile([128, DH], BF, tag="pv",
                                        name="pv")
                        nc.tensor.transpose(
                            pv[:], vT_u[:, kc * 128:(kc + 1) * 128],
                            ident_bf[hrow:hrow + DH, hrow:hrow + DH])
                        nc.scalar.copy(vaug[:, hl, kc, 0:DH], pv[:])
                for j in js:
                    nk = 4 * (j + 1)
                    po = [ps_o.tile([96, 512], F32, tag=f"po{hl}",
                                    name=f"po{hl}") for hl in range(HL)]
                    qwin = slice(tok0 + j * 512, tok0 + (j + 1) * 512)
                    # off-diagonal: full-width pairs, DoubleRow fp8
                    for kp in range((nk - 4) // 2):
                        for hl in range(HL):
                            hrow = hl * DH
                            qsl = qT[hrow:hrow + DH, qwin]
                            est = s2_exp.tile([128, 2, 512], E4,
                                              tag=f"est{hl}",
                                              name=f"est{hl}")
                            for half in range(2):
                                kc = 2 * kp + half
                                ksl = kT[hrow:hrow + DH,
                                         tok0 + kc * 128:
                                         tok0 + (kc + 1) * 128]
                                pst = ps_sc.tile([128, 512], F32,
                                                 tag="pst", name="pst")
                                nc.tensor.matmul(pst[:], ksl, qsl,
                                                 start=True, stop=True,
                                                 tile_position=(hrow, 0))
                                nc.scalar.activation(est[:, half, :],
                                                     pst[:], AF.Exp,
                                                     scale=EXP_SCALE)
                            nc.tensor.matmul(
                                po[hl][:], vaug[:, hl, 2 * kp:2 * kp + 2, :],
                                est[:], start=(kp == 0), stop=False,
                                perf_mode=DR)
                    # diagonal: 4 masked chunks, single fp8 matmuls
                    for dm in range(4):
                        kc = nk - 4 + dm
                        col0 = 128 * dm
                        w = 512 - col0
                        for hl in range(HL):
                            hrow = hl * DH
                            qsl = qT[hrow:hrow + DH,
                                     tok0 + j * 512 + col0:
                                     tok0 + (j + 1) * 512]
                            ksl = kT[hrow:hrow + DH,
                                     tok0 + kc * 128:tok0 + (kc + 1) * 128]
                            pst = ps_sc.tile([128, 512], F32, tag="pst",
                                             name="pst")
                            nc.tensor.matmul(pst[:, :w], ksl, qsl,
                                             start=True, stop=True,
                                             tile_position=(hrow, 0))
                            estd = s2_exp.tile([128, 512], E4,
                                               tag=f"estd{hl}",
                                               name=f"estd{hl}")
                            nc.scalar.activation(estd[:, :w], pst[:, :w],
                                                 AF.Exp, scale=EXP_SCALE)
                            nc.gpsimd.tensor_mul(
                                estd[:, :w], estd[:, :w],
                                masks_sb[dm][:, col0:])
                            nc.tensor.matmul(po[hl][:, col0:],
                                             vaug[:, hl, kc, :],
                                             estd[:, :w],
                                             start=(nk == 4 and dm == 0),
                                             stop=(dm == 3))
                    for hl in range(HL):
                        hrow = hl * DH
                        den = s2_misc.tile([1, 512], F32, tag="den",
                                           name="den")
                        nc.vector.tensor_copy(den[:], po[hl][DH:DH + 1, :])
                        rec1 = s2_misc.tile([1, 512], F32, tag="rec1",
                                            name="rec1")
                        nc.vector.reciprocal_approx_fast(rec1[:], den[:])
                        rec1b = s2_misc.tile([1, 512], BF, tag="rec1b",
                                             name="rec1b")
                        nc.scalar.copy(rec1b[:], rec1[:])
                        rec_b = ps_vt.tile([64, 512], F32, tag="pv",
                                           name="rec_b")
                        nc.tensor.matmul(rec_b[:], ones_row[0:1, 0:64],
                                         rec1b[:], start=True, stop=True)
                        rec_sb = s2_misc.tile([64, 512], BF, tag="rec_sb",
                                              name="rec_sb")
                        nc.vector.tensor_copy(rec_sb[:], rec_b[:])
                        nc.vector.tensor_mul(
                            oT[hrow:hrow + DH, qwin],
                            po[hl][0:DH, :], rec_sb[:])

            def do_oproj(q):
                """out-projection for quarter q (tokens q*1024..+1024) + RS."""
                for tch in range(8):
                    row0 = q * 1024 + tch * 128
                    r1 = s3_r1.tile([128, D], BF, tag="r1", name="r1")
                    for n in range(2):
                        pop = ps_sc.tile([128, 512], F32, tag="pst",
                                         name="pop")
                        nc.tensor.matmul(pop[:], oT[:, row0:row0 + 128],
                                         wout_sb[:, n * 512:(n + 1) * 512],
                                         start=True, stop=True)
                        eng = nc.scalar if n == 0 else nc.vector
                        if n == 0:
                            nc.scalar.copy(
                                r1[:, n * 512:(n + 1) * 512], pop[:])
                        else:
                            nc.vector.tensor_copy(
                                r1[:, n * 512:(n + 1) * 512], pop[:])
                    nc.gpsimd.dma_start(
                        rs1_in[q][tch * 128:(tch + 1) * 128, :], r1[:])
                nc.gpsimd.collective_compute(
                    "ReduceScatter", mybir.AluOpType.add, replica_groups=rg,
                    ins=[rs1_in[q][:].opt()], outs=[rs1_out[q][:].opt()])

            def do_s4(q):
                """residual + LN2 + transpose for my piece of quarter q."""
                r1s = s4_t.tile([128, D], BF, tag="r1s", name="r1s")
                nc.scalar.dma_start(r1s[:], rs1_out[q][:])
                nc.vector.tensor_add(x2_sb[:, q, :], xsv[:, q, :], r1s[:])
                stats = s4_t.tile([128, 2, 6], F32, tag="stats", name="stats")
                x2v = x2_sb[:, q, :].rearrange("p (s f) -> p s f", s=2)
                for s in range(2):
                    nc.vector.bn_stats(stats[:, s, :], x2v[:, s, :])
                mv = s4_t.tile([128, 2], F32, tag="mv", name="mv")
                nc.vector.bn_aggr(mv[:], stats[:])
                rstd2 = s4_t.tile([128, 1], F32, tag="rstd2", name="rstd2")
                nc.scalar.activation(rstd2[:], mv[:, 1:2], AF.Sqrt,
                                     bias=eps128[:])
                nc.vector.reciprocal_approx_fast(rstd2[:], rstd2[:])
                h2 = s4_t.tile([128, D], F32, tag="h2", name="h2")
                nc.vector.tensor_scalar(
                    out=h2[:], in0=x2_sb[:, q, :], scalar1=mv[:, 0:1],
                    scalar2=rstd2[:], op0=mybir.AluOpType.subtract,
                    op1=mybir.AluOpType.mult)
                for d in range(ND):
                    pt = ps_vt.tile([128, 128], F32, tag="pv", name="pt")
                    nc.tensor.transpose(
                        pt[:], h2[:, d * 128:(d + 1) * 128], ident[:])
                    nc.vector.tensor_copy(
                        h2T[:, d, q * 128:(q + 1) * 128], pt[:])

            # ---------------- attention schedule ----------------
            do_attn(0, (0, 1))
            do_oproj(0)
            do_attn(0, (2, 3))
            do_oproj(1)
            do_attn(1, (0, 1))   # hides RS1_0 + RS1_1
            do_oproj(2)
            do_s4(0)
            do_attn(1, (2, 3))
            do_s4(1)
            do_oproj(3)

            # attention-only PSUM pools pop (LIFO)
            ps_o_cm.__exit__(None, None, None)
            ps_sc_cm.__exit__(None, None, None)

            # ---- MLP1 halves (sequence-parallel) ----
            ps_m1_cm = tc.tile_pool(name="ps_m1", bufs=3, space="PSUM")
            ps_m1 = ps_m1_cm.__enter__()

            def do_mlp1(h, ms):
                """gelu(h2 @ w1) for token half h, hidden chunks ms."""
                g1 = g1h[h]
                for m in ms:
                    pm1 = ps_m1.tile([128, 256], F32, tag="pm1",
                                     name="pm1")
                    for d in range(ND):
                        nc.tensor.matmul(
                            pm1[:], w1_sb[:, m, d, :],
                            h2T[:, d, h * 256:(h + 1) * 256],
                            start=(d == 0), stop=(d == ND - 1))
                    nc.scalar.activation(g1[:, m, :], pm1[:], AF.Gelu,
                                         bias=b1g_sb[:, m:m + 1])

            do_mlp1(0, range(12))        # needs s4(0), s4(1)
            do_s4(2)
            do_mlp1(0, range(12, 24))
            do_s4(3)
            do_mlp1(0, range(24, NM))
            do_mlp1(1, range(NM))

            ps_m1_cm.__exit__(None, None, None)
            ps_vt_cm.__exit__(None, None, None)
            for cm in (s4_t_cm, s3_r1_cm, s2_misc_cm, s2_exp_cm,
                       s2_vaug_cm, b1g_pool_cm, w1_pool_cm):
                cm.__exit__(None, None, None)

            # ---- MLP2 with streamed w2 (all 4 quarters at once) ----
            s6_w2_cm = tc.tile_pool(name="s6_w2", bufs=2)
            s6_w2 = s6_w2_cm.__enter__()
            s7_cm = tc.tile_pool(name="s7", bufs=2)
            s7 = s7_cm.__enter__()
            ps_m2_cm = tc.tile_pool(name="ps_m2", bufs=1, space="PSUM")
            ps_m2 = ps_m2_cm.__enter__()

            pm2 = [[ps_m2.tile([128, 512], F32, tag=f"pm2_{q}_{n}",
                               name=f"pm2_{q}_{n}") for n in range(2)]
                   for q in range(4)]
            w2s_t = {}

            def fetch_w2(m2):
                w2s_t[m2] = s6_w2.tile([128, 4, D], BF, tag="w2s",
                                       name="w2s")
                nc.sync.dma_start(w2s_t[m2][:],
                                  w2_in.ap()[:, 4 * m2:4 * (m2 + 1), :])

            def finalize(q):
                ot = s7.tile([128, D], F32, tag="ot", name="ot")
                for n in range(2):
                    nc.vector.tensor_add(
                        ot[:, n * 512:(n + 1) * 512],
                        x2_sb[:, q, n * 512:(n + 1) * 512], pm2[q][n][:])
                nc.sync.dma_start(
                    out_ext.ap()[q * 128:(q + 1) * 128, :], ot[:])

            fetch_w2(0)
            fetch_w2(1)
            for m2 in range(7):
                if m2 + 2 < 8:
                    fetch_w2(m2 + 2)
                w2s = w2s_t[m2]
                for mi in range(4):
                    m = 4 * m2 + mi
                    for q in range(4):
                        g1 = g1h[q // 2]
                        tch = q % 2
                        for n in range(2):
                            nc.tensor.matmul(
                                pm2[q][n][:],
                                g1[:, m, tch * 128:(tch + 1) * 128],
                                w2s[:, mi, n * 512:(n + 1) * 512],
                                start=(m == 0), stop=False)
                # last chunk: finish q-major so outputs stream out early
            w2s = w2s_t[7]
            for q in range(4):
                g1 = g1h[q // 2]
                tch = q % 2
                for mi in range(4):
                    m = 28 + mi
                    for n in range(2):
                        nc.tensor.matmul(
                            pm2[q][n][:],
                            g1[:, m, tch * 128:(tch + 1) * 128],
                            w2s[:, mi, n * 512:(n + 1) * 512],
                            start=False, stop=(m == NM - 1))
                finalize(q)

            ps_m2_cm.__exit__(None, None, None)
            for cm in (s7_cm, s6_w2_cm, attn_pool_cm, xsv_pool_cm,
                       resid_pool_cm):
                cm.__exit__(None, None, None)

    nc.compile()
    _CACHE["nc"] = nc
    return nc


def shard_rows(c):
    """Global token rows owned by core c (four strided pieces of 128)."""
    return np.concatenate(
        [np.arange(q * 1024 + c * 128, q * 1024 + (c + 1) * 128)
         for q in range(4)])


def _fp8(a, scale):
    import ml_dtypes
    return np.clip(np.asarray(a, np.float32) * scale,
                   -240.0, 240.0).astype(ml_dtypes.float8_e4m3)


def make_in_maps(x, ln1_g, ln1_b, w_qkv, w_out, ln2_g, ln2_b, w1, b1, w2, b2):
    import ml_dtypes
    bf16 = ml_dtypes.bfloat16
    x = np.asarray(x, np.float32)
    xf = np.ascontiguousarray(x.reshape(T, D))
    xt8 = np.ascontiguousarray(_fp8(xf.T, S_X).reshape(ND, 128, T))
    import ml_dtypes as _md
    xbf = np.ascontiguousarray(xf.astype(_md.bfloat16))
    w_qkv_eff = np.asarray(w_qkv) * np.asarray(ln1_g)[:, None]
    bias_qkv = np.asarray(ln1_b) @ np.asarray(w_qkv)
    assert np.abs(bias_qkv).max() == 0.0, "nonzero qkv bias unsupported"
    assert np.abs(np.asarray(b2)).max() == 0.0, "nonzero b2 unsupported"
    w1_eff = np.asarray(w1) * np.asarray(ln2_g)[:, None]
    bias_h1 = np.asarray(ln2_b) @ np.asarray(w1) + np.asarray(b1)
    km = np.arange(128)[:, None]
    qm = np.arange(512)[None, :]
    masks = np.stack([(km + 128 * m <= qm).astype(ml_dtypes.float8_e4m3)
                      for m in range(4)])
    w1h = np.ascontiguousarray(
        w1_eff.astype(bf16).reshape(ND, 128, NM, 128)
        .transpose(1, 2, 0, 3))
    b1gh = np.ascontiguousarray(
        np.asarray(bias_h1, np.float32).reshape(NM, 128).T)
    w2h = np.ascontiguousarray(
        np.asarray(w2).astype(bf16).reshape(NM, 128, D).transpose(1, 0, 2))
    in_maps = []
    for c in range(NCORES):
        cs = slice(c * DLOC, (c + 1) * DLOC)
        wq = np.concatenate(
            [w_qkv_eff[:, cs], w_qkv_eff[:, D:][:, cs],
             w_qkv_eff[:, 2 * D:][:, cs]], axis=1)
        wq8 = _fp8(wq, S_W)                      # [1024, 384]
        nws = wq8.astype(np.float32).sum(axis=0) / S_W      # [384]
        wq8p = np.ascontiguousarray(
            wq8.reshape(4, 2, 128, 3 * DLOC).transpose(2, 0, 1, 3))
        rows = shard_rows(c)
        in_maps.append({
            "xt": xt8,
            "xbf": xbf,
            "xsh": np.ascontiguousarray(xf[rows].astype(bf16)),
            "wqkv": wq8p,
            "nws": np.ascontiguousarray(
                nws.reshape(3, 128).T.astype(np.float32)),
            "wout": np.ascontiguousarray(
                np.asarray(w_out)[cs].astype(bf16)),
            "w1": w1h,
            "b1g": b1gh,
            "w2": w2h,
            "masks": masks,
        })
    return in_maps


def kernel(**inputs):
    nc = build()
    in_maps = make_in_maps(**inputs)
    res = bass_utils.run_bass_kernel_spmd(
        nc, in_maps, core_ids=list(range(NCORES)))
    out = np.empty((T, D), np.float32)
    for c in range(NCORES):
        out[shard_rows(c)] = res.results[c]["out"]
    return out.reshape(B, L, D).astype(np.float32)


# revision 37
# speedup vs baseline: 1.0131x; 1.0131x over previous
"""Trainium2 8-core kernel for a dense pre-norm transformer block.

Reference: h=LN1(x); qkv=h@w_qkv; causal MHA (16 heads, Dh=64);
x+=o@w_out; h2=LN2(x); x+=gelu(h2@w1+b1)@w2+b2.

Sharding:
  - Attention Megatron-TP8: heads 2c,2c+1 on core c (w_qkv column-shard,
    w_out row-shard), ReduceScatter of the out-projection in 4 token
    quarters (core c owns strided pieces {q*1024 + c*128 .. +128}).
  - MLP sequence-parallel: every core holds FULL w1/w2 and runs the MLP
    only for its own 512 tokens.  No AllGather, no second
    ReduceScatter; RS1 is the only large collective.  w1 is resident,
    w2 is streamed in 2MB chunks during MLP2.
  - LN1 stats shard-local (bn_stats on own tokens) + tiny AllGather;
    LN mean-subtraction folded into the qkv matmul as a rank-1 update;
    gains/biases folded host-side.

Precision: fp8(e4m3) DoubleRow matmuls for qkv (x scaled x8, w x512),
fp8 scores + fp8 AV (q,k,v stored x16; off-diagonal k-chunk pairs run
DoubleRow, causal-diagonal chunks run single fp8 matmuls with masks).
Out-projection and both MLP matmuls stay bf16.  fp32 PSUM everywhere;
fp32 residual stream.
"""
import numpy as np

import concourse.bass as bass
import concourse.mybir as mybir
import concourse.tile as tile
from concourse import bacc
from concourse import bass_utils
from concourse.masks import make_identity

F32 = mybir.dt.float32
BF = mybir.dt.bfloat16
E4 = mybir.dt.float8e4
AF = mybir.ActivationFunctionType
DR = mybir.MatmulPerfMode.DoubleRow

NCORES = 8
B, L, D = 2, 2048, 1024
T = B * L              # 4096 tokens
TSH = T // NCORES      # 512 tokens per core (4 pieces of 128)
DH = 64                # head dim
HL = 2                 # heads per core
DLOC = HL * DH         # 128 local head features
LN_EPS = 1e-5
NT = T // 512          # 8 token tiles of 512
ND = D // 128          # 8 feature chunks
MH = 4096              # full MLP hidden
NM = MH // 128         # 32 hidden chunks

S_X = 8.0              # fp8 scale on x
S_W = 512.0            # fp8 scale on w_qkv
QS = 16.0              # q,k,v storage scale
# exp(scale * pst): pst = (16q).(16k), ref scale 1/8 -> 1/(8*256)
EXP_SCALE = 0.125 / (QS * QS)
# u-term: praw_psum = (S_X*S_W) * x@w ; want 16*praw*rstd
K16S = QS / (S_X * S_W)   # 2^-8, exact in bf16

_CACHE = {}


def build():
    if "nc" in _CACHE:
        return _CACHE["nc"]
    nc = bacc.Bacc("TRN2", target_bir_lowering=False, debug=False,
                   num_devices=NCORES)

    xt_in = nc.dram_tensor("xt", [ND, 128, T], E4, kind="ExternalInput")
    xsh_in = nc.dram_tensor("xsh", [TSH, D], BF, kind="ExternalInput")
    xbf_in = nc.dram_tensor("xbf", [T, D], BF, kind="ExternalInput")
    wqkv_in = nc.dram_tensor("wqkv", [128, 4, 2, 3 * DLOC], E4,
                             kind="ExternalInput")
    nws_in = nc.dram_tensor("nws", [128, 3], F32, kind="ExternalInput")
    wout_in = nc.dram_tensor("wout", [DLOC, D], BF, kind="ExternalInput")
    w1_in = nc.dram_tensor("w1", [128, NM, ND, 128], BF,
                           kind="ExternalInput")
    b1g_in = nc.dram_tensor("b1g", [128, NM], F32, kind="ExternalInput")
    w2_in = nc.dram_tensor("w2", [128, NM, D], BF, kind="ExternalInput")
    masks_in = nc.dram_tensor("masks", [4, 128, 512], E4,
                              kind="ExternalInput")
    out_ext = nc.dram_tensor("out", [TSH, D], F32, kind="ExternalOutput")

    rg = [list(range(NCORES))]

    with tile.TileContext(nc) as tc:
        with (
            tc.tile_pool(name="const", bufs=1) as const,
            tc.tile_pool(name="dram", bufs=1, space="DRAM") as dram,
        ):
            # ---- DRAM scratch for collectives ----
            st_ag_in = dram.tile([8, 128], BF)       # (piece q, mean/rstd)
            st_ag_out = dram.tile([64, 128], BF, addr_space="Shared")
            rs1_in = [dram.tile([1024, D], BF, name=f"rs1_in{q}")
                      for q in range(4)]
            rs1_out = [dram.tile([128, D], BF, name=f"rs1_out{q}")
                       for q in range(4)]
            warm_in = dram.tile([8, 16], BF)
            warm_out = dram.tile([64, 16], BF, addr_space="Shared")

            # ---- constants ----
            ident = const.tile([128, 128], F32)
            make_identity(nc, ident[:])
            ident_bf = const.tile([128, 128], BF)
            make_identity(nc, ident_bf[:])
            ones_row = const.tile([1, 128], BF)
            nc.vector.memset(ones_row[:], 1.0)
            k16s_row = const.tile([1, 128], BF)
            nc.vector.memset(k16s_row[:], K16S)
            neg16_row = const.tile([1, 128], BF)
            nc.vector.memset(neg16_row[:], -QS)
            eps128 = const.tile([128, 1], F32)
            nc.vector.memset(eps128[:], LN_EPS)

            # residual stream + h2^T live to the very end
            resid_pool_cm = tc.tile_pool(name="resid", bufs=1)
            resid_pool = resid_pool_cm.__enter__()
            x2_sb = resid_pool.tile([128, 4, D], BF)
            h2T = resid_pool.tile([128, ND, TSH], BF)
            g1h = [resid_pool.tile([128, NM, 256], BF, name=f"g1_{h}")
                   for h in range(2)]


            # x shard, preloaded early (residual adds in s4)
            xsv_pool_cm = tc.tile_pool(name="xsv", bufs=1)
            xsv_pool = xsv_pool_cm.__enter__()
            xsv = xsv_pool.tile([128, 4, D], BF)

            # attention activations + small weights: die after oproj(3)
            attn_pool_cm = tc.tile_pool(name="attnp", bufs=1)
            attn_pool = attn_pool_cm.__enter__()
            qT = attn_pool.tile([128, T], E4)     # 16*q
            kT = attn_pool.tile([128, T], E4)     # 16*k
            vT = attn_pool.tile([128, T], BF)     # 16*v
            oT = attn_pool.tile([128, T], BF)     # o (exact)
            masks_sb = [attn_pool.tile([128, 512], E4, name=f"mask{m}")
                        for m in range(4)]
            wout_sb = attn_pool.tile([DLOC, D], BF)
            nws_sb = attn_pool.tile([128, 3], F32)
            for m in range(4):
                nc.scalar.dma_start(masks_sb[m][:], masks_in.ap()[m])
            nc.scalar.dma_start(nws_sb[:], nws_in.ap())

            # ========== stage 0: warm collective + local LN1 stats ======
            # per-token LN1 stats, one row each (partition base 0)
            mean_sb = attn_pool.tile([1, T // 128, 128], BF)
            rstd_sb = attn_pool.tile([1, T // 128, 128], BF)

            s0_cm = tc.tile_pool(name="s0", bufs=4)
            s0 = s0_cm.__enter__()
            ps0_cm = tc.tile_pool(name="ps0", bufs=2, space="PSUM")
            ps0 = ps0_cm.__enter__()
            ps_st_cm = tc.tile_pool(name="ps_st", bufs=2, space="PSUM")
            ps_st = ps_st_cm.__enter__()
            # absorb first-collective init latency with a no-dep dummy
            wtile = s0.tile([8, 16], BF, tag="wtile", name="wtile")
            nc.vector.memset(wtile[:], 0.0)
            nc.scalar.dma_start(warm_in[:], wtile[:])
            nc.gpsimd.collective_compute(
                "AllGather", mybir.AluOpType.bypass, replica_groups=rg,
                ins=[warm_in[:].opt()], outs=[warm_out[:].opt()])


            def do_stats(tc_):
                """LN1 stats for token chunk tc_ (128 tokens), all local."""
                eng = nc.vector
                xb = s0.tile([128, D], BF, tag="xb", name="xb")
                nc.scalar.dma_start(
                    xb[:], xbf_in.ap()[tc_ * 128:(tc_ + 1) * 128, :])
                stats = s0.tile([128, 2, 6], F32, tag="stats", name="stats")
                xv = xb[:].rearrange("p (s f) -> p s f", s=2)
                for s in range(2):
                    eng.bn_stats(stats[:, s, :], xv[:, s, :])
                mv = s0.tile([128, 2], F32, tag="mv", name="mv")
                eng.bn_aggr(mv[:], stats[:])
                rstd0 = s0.tile([128, 1], F32, tag="rstd0", name="rstd0")
                nc.scalar.activation(rstd0[:], mv[:, 1:2], AF.Sqrt,
                                     bias=eps128[:])
                nc.vector.reciprocal_approx_fast(rstd0[:], rstd0[:])
                stp_m = ps0.tile([1, 128], F32, tag="stp", name="stp_m")
                nc.tensor.transpose(stp_m[:], mv[:, 0:1], ident[:])
                nc.scalar.copy(mean_sb[:, tc_, :], stp_m[:])
                stp_r = ps0.tile([1, 128], F32, tag="stp", name="stp_r")
                nc.tensor.transpose(stp_r[:], rstd0[:], ident[:])
                nc.scalar.copy(rstd_sb[:, tc_, :], stp_r[:])

            # ================= stage 1: LN1 + qkv (fp8 DoubleRow) ========
            s1_tmp_cm = tc.tile_pool(name="s1_tmp", bufs=3)
            s1_tmp = s1_tmp_cm.__enter__()
            s1_stat_cm = tc.tile_pool(name="s1_stat", bufs=1)
            s1_stat = s1_stat_cm.__enter__()
            praw_pool_cm = tc.tile_pool(name="s1_praw", bufs=12)
            praw_pool = praw_pool_cm.__enter__()
            s1_x_cm = tc.tile_pool(name="s1_x", bufs=1)
            s1_x = s1_x_cm.__enter__()
            ps_qkv_cm = tc.tile_pool(name="ps_qkv", bufs=3, space="PSUM")
            ps_qkv = ps_qkv_cm.__enter__()
            praws = {}

            # whole x^T resident in fp8: [128, d-chunk, tok]
            wqkv_sb = s1_x.tile([128, 4, 2, 3 * DLOC], E4)
            nc.sync.dma_start(wqkv_sb[:], wqkv_in.ap())
            xts_p = [s1_x.tile([128, 2, T], E4, name=f"xts{p}")
                     for p in range(4)]
            for d in range(ND):
                nc.sync.dma_start(xts_p[d // 2][:, d % 2, :],
                                  xt_in.ap()[d])
            for q in range(4):
                nc.sync.dma_start(
                    xsv[:, q, :], xsh_in.ap()[q * 128:(q + 1) * 128, :])
            nc.sync.dma_start(wout_sb[:], wout_in.ap())

            def do_s1_mm(tt):
                praw3 = []
                for m in range(3):
                    ps_q = ps_qkv.tile([128, 512], F32, tag="ps_q",
                                       name="ps_q")
                    for p in range(4):
                        nc.tensor.matmul(
                            ps_q[:],
                            wqkv_sb[:, p, :, m * 128:(m + 1) * 128],
                            xts_p[p][:, :, tt * 512:(tt + 1) * 512],
                            start=(p == 0), stop=(p == 3),
                            perf_mode=DR)
                    praw = praw_pool.tile([128, 512], BF, tag="praw",
                                          name="praw")
                    nc.scalar.copy(praw[:], ps_q[:])
                    praw3.append(praw)
                praws[tt] = praw3

            def do_s1_fin(tt):
                praw3 = praws[tt]
                mean_v = mean_sb[0:1, 4 * tt:4 * tt + 4, :].rearrange(
                    "p a f -> p (a f)")
                rstd_v = rstd_sb[0:1, 4 * tt:4 * tt + 4, :].rearrange(
                    "p a f -> p (a f)")
                mr = s1_stat.tile([1, 512], BF, tag="mr", name="mr")
                nc.vector.tensor_mul(mr[:], mean_v, rstd_v)
                # rstd_b = 2^-8 * rstd, broadcast to 128 partitions
                rstd_b = ps_st.tile([128, 512], F32, tag="pst",
                                    name="rstd_b")
                nc.tensor.matmul(rstd_b[:], k16s_row[:], rstd_v,
                                 start=True, stop=True)
                # mr_b = -16 * mean * rstd, broadcast
                mr_b = ps_st.tile([128, 512], F32, tag="pst", name="mr_b")
                nc.tensor.matmul(mr_b[:], neg16_row[:], mr[:],
                                 start=True, stop=True)
                outs = (qT, kT, vT)
                for m in (2, 0, 1):
                    u = s1_tmp.tile([128, 512], BF, tag="pre", name="u")
                    nc.vector.tensor_mul(u[:], praw3[m][:], rstd_b[:])
                    nc.vector.scalar_tensor_tensor(
                        out=outs[m][:, tt * 512:(tt + 1) * 512],
                        in0=mr_b[:], scalar=nws_sb[:, m:m + 1],
                        in1=u[:], op0=mybir.AluOpType.mult,
                        op1=mybir.AluOpType.add)
                del praws[tt]

            # ---------------- stage-1 schedule ----------------
            for tc_ in range(12):
                do_stats(tc_)
            do_s1_mm(0)
            do_s1_mm(1)
            do_s1_mm(2)
            do_s1_fin(0)
            for tt in range(3, 8):
                do_s1_mm(tt)
                for tc_ in range(4 * tt, 4 * tt + 4):
                    do_stats(tc_)
                do_s1_fin(tt - 2)
            do_s1_fin(6)
            do_s1_fin(7)

            ps_qkv_cm.__exit__(None, None, None)
            ps_st_cm.__exit__(None, None, None)
            ps0_cm.__exit__(None, None, None)
            s1_x_cm.__exit__(None, None, None)   # frees xts + wqkv
            praw_pool_cm.__exit__(None, None, None)
            s1_stat_cm.__exit__(None, None, None)
            s1_tmp_cm.__exit__(None, None, None)
            s0_cm.__exit__(None, None, None)

            # MLP w1 resident; streams in under attention
            w1_pool_cm = tc.tile_pool(name="w1pool", bufs=1)
            w1_pool = w1_pool_cm.__enter__()
            b1g_pool_cm = tc.tile_pool(name="b1gpool", bufs=1)
            b1g_pool = b1g_pool_cm.__enter__()
            b1g_sb = b1g_pool.tile([128, NM], F32)
            nc.scalar.dma_start(b1g_sb[:], b1g_in.ap())
            w1_sb = w1_pool.tile([128, NM, ND, 128], BF)
            for m2 in range(8):
                nc.sync.dma_start(
                    w1_sb[:, 4 * m2:4 * (m2 + 1), :, :],
                    w1_in.ap()[:, 4 * m2:4 * (m2 + 1), :, :])

            # ============ attention pools ============
            s2_vaug_cm = tc.tile_pool(name="s2_vaug", bufs=1)
            s2_vaug = s2_vaug_cm.__enter__()
            s2_exp_cm = tc.tile_pool(name="s2_exp", bufs=2)
            s2_exp = s2_exp_cm.__enter__()
            s2_misc_cm = tc.tile_pool(name="s2_misc", bufs=2)
            s2_misc = s2_misc_cm.__enter__()
            s3_r1_cm = tc.tile_pool(name="s3_r1", bufs=3)
            s3_r1 = s3_r1_cm.__enter__()
            s4_t_cm = tc.tile_pool(name="s4_t", bufs=2)
            s4_t = s4_t_cm.__enter__()
            ps_vt_cm = tc.tile_pool(name="ps_vt", bufs=2, space="PSUM")
            ps_vt = ps_vt_cm.__enter__()
            ps_sc_cm = tc.tile_pool(name="ps_sc", bufs=3, space="PSUM")
            ps_sc = ps_sc_cm.__enter__()
            ps_o_cm = tc.tile_pool(name="ps_o", bufs=1, space="PSUM")
            ps_o = ps_o_cm.__enter__()

            vaug_cur = [None]

            def do_attn(b, js):
                tok0 = b * L
                if js[0] == 0:
                    # slot padded to 96: dual-fp8 ldweights needs the
                    # pair stride to be a multiple of 32
                    vaug = s2_vaug.tile([128, HL, L // 128, 96], E4,
                                        tag="vaug", name="vaug")
                    nc.vector.memset(vaug[:, :, :, DH:96], 0.0)
                    # denominator column: scale cancels against 16*v
                    nc.vector.memset(vaug[:, :, :, DH:DH + 1], QS)
                    vaug_cur[0] = vaug
                    for hl in range(HL):
                        hrow = hl * DH
                        vT_u = vT[hrow:hrow + DH, tok0:tok0 + L]
                        for kc in range(L // 128):
                            pv = ps_vt.tile([128, DH], BF, tag="pv",
                                            name="pv")
                            nc.tensor.transpose(
                                pv[:], vT_u[:, kc * 128:(kc + 1) * 128],
                                ident_bf[hrow:hrow + DH, hrow:hrow + DH])
                            nc.scalar.copy(vaug[:, hl, kc, 0:DH], pv[:])
                vaug = vaug_cur[0]
                for j in js:
                    nk = 4 * (j + 1)
                    po = [ps_o.tile([96, 512], F32, tag=f"po{hl}",
                                    name=f"po{hl}") for hl in range(HL)]
                    qwin = slice(tok0 + j * 512, tok0 + (j + 1) * 512)
                    # off-diagonal: full-width pairs, DoubleRow fp8
                    for kp in range((nk - 4) // 2):
                        for hl in range(HL):
                            hrow = hl * DH
                            qsl = qT[hrow:hrow + DH, qwin]
                            est = s2_exp.tile([128, 2, 512], E4,
                                              tag=f"est{hl}",
                                              name=f"est{hl}")
                            for half in range(2):
                                kc = 2 * kp + half
                                ksl = kT[hrow:hrow + DH,
                                         tok0 + kc * 128:
                                         tok0 + (kc + 1) * 128]
                                pst = ps_sc.tile([128, 512], F32,
                                                 tag="pst", name="pst")
                                nc.tensor.matmul(pst[:], ksl, qsl,
                                                 start=True, stop=True,
                                                 tile_position=(hrow, 0))
                                nc.scalar.activation(est[:, half, :],
                                                     pst[:], AF.Exp,
                                                     scale=EXP_SCALE)
                            nc.tensor.matmul(
                                po[hl][:], vaug[:, hl, 2 * kp:2 * kp + 2, :],
                                est[:], start=(kp == 0), stop=False,
                                perf_mode=DR)
                    # diagonal: 4 masked chunks, single fp8 matmuls
                    for dm in range(4):
                        kc = nk - 4 + dm
                        col0 = 128 * dm
                        w = 512 - col0
                        for hl in range(HL):
                            hrow = hl * DH
                            qsl = qT[hrow:hrow + DH,
                                     tok0 + j * 512 + col0:
                                     tok0 + (j + 1) * 512]
                            ksl = kT[hrow:hrow + DH,
                                     tok0 + kc * 128:tok0 + (kc + 1) * 128]
                            pst = ps_sc.tile([128, 512], F32, tag="pst",
                                             name="pst")
                            nc.tensor.matmul(pst[:, :w], ksl, qsl,
                                             start=True, stop=True,
                                             tile_position=(hrow, 0))
                            estd = s2_exp.tile([128, 512], E4,
                                               tag=f"estd{hl}",
                                               name=f"estd{hl}")
                            nc.scalar.activation(estd[:, :w], pst[:, :w],
                                                 AF.Exp, scale=EXP_SCALE)
                            nc.gpsimd.tensor_mul(
                                estd[:, :w], estd[:, :w],
                                masks_sb[dm][:, col0:])
                            nc.tensor.matmul(po[hl][:, col0:],
                                             vaug[:, hl, kc, :],
                                             estd[:, :w],
                                             start=(nk == 4 and dm == 0),
                                             stop=(dm == 3))
                    for hl in range(HL):
                        hrow = hl * DH
                        den = s2_misc.tile([1, 512], F32, tag="den",
                                           name="den")
                        nc.vector.tensor_copy(den[:], po[hl][DH:DH + 1, :])
                        rec1 = s2_misc.tile([1, 512], F32, tag="rec1",
                                            name="rec1")
                        nc.vector.reciprocal_approx_fast(rec1[:], den[:])
                        rec1b = s2_misc.tile([1, 512], BF, tag="rec1b",
                                             name="rec1b")
                        nc.scalar.copy(rec1b[:], rec1[:])
                        rec_b = ps_vt.tile([64, 512], F32, tag="pv",
                                           name="rec_b")
                        nc.tensor.matmul(rec_b[:], ones_row[0:1, 0:64],
                                         rec1b[:], start=True, stop=True)
                        rec_sb = s2_misc.tile([64, 512], BF, tag="rec_sb",
                                              name="rec_sb")
                        nc.vector.tensor_copy(rec_sb[:], rec_b[:])
                        nc.vector.tensor_mul(
                            oT[hrow:hrow + DH, qwin],
                            po[hl][0:DH, :], rec_sb[:])

            def do_oproj(q):
                """out-projection for quarter q (tokens q*1024..+1024) + RS."""
                for tch in range(8):
                    row0 = q * 1024 + tch * 128
                    r1 = s3_r1.tile([128, D], BF, tag="r1", name="r1")
                    for n in range(2):
                        pop = ps_sc.tile([128, 512], F32, tag="pst",
                                         name="pop")
                        nc.tensor.matmul(pop[:], oT[:, row0:row0 + 128],
                                         wout_sb[:, n * 512:(n + 1) * 512],
                                         start=True, stop=True)
                        eng = nc.scalar if n == 0 else nc.vector
                        if n == 0:
                            nc.scalar.copy(
                                r1[:, n * 512:(n + 1) * 512], pop[:])
                        else:
                            nc.vector.tensor_copy(
                                r1[:, n * 512:(n + 1) * 512], pop[:])
                    nc.gpsimd.dma_start(
                        rs1_in[q][tch * 128:(tch + 1) * 128, :], r1[:])
                nc.gpsimd.collective_compute(
                    "ReduceScatter", mybir.AluOpType.add, replica_groups=rg,
                    ins=[rs1_in[q][:].opt()], outs=[rs1_out[q][:].opt()])

            def do_s4(q):
                """residual + LN2 + transpose for my piece of quarter q."""
                r1s = s4_t.tile([128, D], BF, tag="r1s", name="r1s")
                nc.scalar.dma_start(r1s[:], rs1_out[q][:])
                nc.vector.tensor_add(x2_sb[:, q, :], xsv[:, q, :], r1s[:])
                stats = s4_t.tile([128, 2, 6], F32, tag="stats", name="stats")
                x2v = x2_sb[:, q, :].rearrange("p (s f) -> p s f", s=2)
                for s in range(2):
                    nc.vector.bn_stats(stats[:, s, :], x2v[:, s, :])
                mv = s4_t.tile([128, 2], F32, tag="mv", name="mv")
                nc.vector.bn_aggr(mv[:], stats[:])
                rstd2 = s4_t.tile([128, 1], F32, tag="rstd2", name="rstd2")
                nc.scalar.activation(rstd2[:], mv[:, 1:2], AF.Sqrt,
                                     bias=eps128[:])
                nc.vector.reciprocal_approx_fast(rstd2[:], rstd2[:])
                h2 = s4_t.tile([128, D], F32, tag="h2", name="h2")
                nc.vector.tensor_scalar(
                    out=h2[:], in0=x2_sb[:, q, :], scalar1=mv[:, 0:1],
                    scalar2=rstd2[:], op0=mybir.AluOpType.subtract,
                    op1=mybir.AluOpType.mult)
                for d in range(ND):
                    pt = ps_vt.tile([128, 128], F32, tag="pv", name="pt")
                    nc.tensor.transpose(
                        pt[:], h2[:, d * 128:(d + 1) * 128], ident[:])
                    nc.vector.tensor_copy(
                        h2T[:, d, q * 128:(q + 1) * 128], pt[:])

            # ---------------- attention schedule ----------------
            do_attn(0, (0, 1))
            do_oproj(0)
            do_attn(0, (2, 3))
            do_oproj(1)
            do_attn(1, (0, 1))   # hides RS1_0 + RS1_1
            do_oproj(2)
            do_s4(0)
            do_attn(1, (2, 3))
            do_s4(1)
            do_oproj(3)

            # attention-only PSUM pools pop (LIFO)
            ps_o_cm.__exit__(None, None, None)
            ps_sc_cm.__exit__(None, None, None)

            # ---- MLP1 halves (sequence-parallel) ----
            ps_m1_cm = tc.tile_pool(name="ps_m1", bufs=3, space="PSUM")
            ps_m1 = ps_m1_cm.__enter__()

            def do_mlp1(h, ms):
                """gelu(h2 @ w1) for token half h, hidden chunks ms."""
                g1 = g1h[h]
                for m in ms:
                    pm1 = ps_m1.tile([128, 256], F32, tag="pm1",
                                     name="pm1")
                    for d in range(ND):
                        nc.tensor.matmul(
                            pm1[:], w1_sb[:, m, d, :],
                            h2T[:, d, h * 256:(h + 1) * 256],
                            start=(d == 0), stop=(d == ND - 1))
                    nc.scalar.activation(g1[:, m, :], pm1[:], AF.Gelu,
                                         bias=b1g_sb[:, m:m + 1])

            do_mlp1(0, range(12))        # needs s4(0), s4(1)
            do_s4(2)
            do_mlp1(0, range(12, 24))
            do_s4(3)
            do_mlp1(0, range(24, NM))
            do_mlp1(1, range(NM))

            ps_m1_cm.__exit__(None, None, None)
            ps_vt_cm.__exit__(None, None, None)
            for cm in (s4_t_cm, s3_r1_cm, s2_misc_cm, s2_exp_cm,
                       s2_vaug_cm, b1g_pool_cm, w1_pool_cm):
                cm.__exit__(None, None, None)

            # ---- MLP2 with streamed w2 (all 4 quarters at once) ----
            s6_w2_cm = tc.tile_pool(name="s6_w2", bufs=2)
            s6_w2 = s6_w2_cm.__enter__()
            s7_cm = tc.tile_pool(name="s7", bufs=2)
            s7 = s7_cm.__enter__()
            ps_m2_cm = tc.tile_pool(name="ps_m2", bufs=1, space="PSUM")
            ps_m2 = ps_m2_cm.__enter__()

            pm2 = [[ps_m2.tile([128, 512], F32, tag=f"pm2_{q}_{n}",
                               name=f"pm2_{q}_{n}") for n in range(2)]
                   for q in range(4)]
            w2s_t = {}

            def fetch_w2(m2):
                w2s_t[m2] = s6_w2.tile([128, 4, D], BF, tag="w2s",
                                       name="w2s")
                nc.sync.dma_start(w2s_t[m2][:],
                                  w2_in.ap()[:, 4 * m2:4 * (m2 + 1), :])

            def finalize(q):
                ot = s7.tile([128, D], F32, tag="ot", name="ot")
                for n in range(2):
                    nc.vector.tensor_add(
                        ot[:, n * 512:(n + 1) * 512],
                        x2_sb[:, q, n * 512:(n + 1) * 512], pm2[q][n][:])
                nc.sync.dma_start(
                    out_ext.ap()[q * 128:(q + 1) * 128, :], ot[:])

            fetch_w2(0)
            fetch_w2(1)
            for m2 in range(7):
                if m2 + 2 < 8:
                    fetch_w2(m2 + 2)
                w2s = w2s_t[m2]
                for mi in range(4):
                    m = 4 * m2 + mi
                    for q in range(4):
                        g1 = g1h[q // 2]
                        tch = q % 2
                        for n in range(2):
                            nc.tensor.matmul(
                                pm2[q][n][:],
                                g1[:, m, tch * 128:(tch + 1) * 128],
                                w2s[:, mi, n * 512:(n + 1) * 512],
                                start=(m == 0), stop=False)
                # last chunk: finish q-major so outputs stream out early
            w2s = w2s_t[7]
            for q in range(4):
                g1 = g1h[q // 2]
                tch = q % 2
                for mi in range(4):
                    m = 28 + mi
                    for n in range(2):
                        nc.tensor.matmul(
                            pm2[q][n][:],
                            g1[:, m, tch * 128:(tch + 1) * 128],
                            w2s[:, mi, n * 512:(n + 1) * 512],
                            start=False, stop=(m == NM - 1))
                finalize(q)

            ps_m2_cm.__exit__(None, None, None)
            for cm in (s7_cm, s6_w2_cm, attn_pool_cm, xsv_pool_cm,
                       resid_pool_cm):
                cm.__exit__(None, None, None)

    nc.compile()
    _CACHE["nc"] = nc
    return nc


def shard_rows(c):
    """Global token rows owned by core c (four strided pieces of 128)."""
    return np.concatenate(
        [np.arange(q * 1024 + c * 128, q * 1024 + (c + 1) * 128)
         for q in range(4)])


def _fp8(a, scale):
    import ml_dtypes
    return np.clip(np.asarray(a, np.float32) * scale,
                   -240.0, 240.0).astype(ml_dtypes.float8_e4m3)


def make_in_maps(x, ln1_g, ln1_b, w_qkv, w_out, ln2_g, ln2_b, w1, b1, w2, b2):
    import ml_dtypes
    bf16 = ml_dtypes.bfloat16
    x = np.asarray(x, np.float32)
    xf = np.ascontiguousarray(x.reshape(T, D))
    xt8 = np.ascontiguousarray(_fp8(xf.T, S_X).reshape(ND, 128, T))
    import ml_dtypes as _md
    xbf = np.ascontiguousarray(xf.astype(_md.bfloat16))
    w_qkv_eff = np.asarray(w_qkv) * np.asarray(ln1_g)[:, None]
    bias_qkv = np.asarray(ln1_b) @ np.asarray(w_qkv)
    assert np.abs(bias_qkv).max() == 0.0, "nonzero qkv bias unsupported"
    assert np.abs(np.asarray(b2)).max() == 0.0, "nonzero b2 unsupported"
    w1_eff = np.asarray(w1) * np.asarray(ln2_g)[:, None]
    bias_h1 = np.asarray(ln2_b) @ np.asarray(w1) + np.asarray(b1)
    km = np.arange(128)[:, None]
    qm = np.arange(512)[None, :]
    masks = np.stack([(km + 128 * m <= qm).astype(ml_dtypes.float8_e4m3)
                      for m in range(4)])
    w1h = np.ascontiguousarray(
        w1_eff.astype(bf16).reshape(ND, 128, NM, 128)
        .transpose(1, 2, 0, 3))
    b1gh = np.ascontiguousarray(
        np.asarray(bias_h1, np.float32).reshape(NM, 128).T)
    w2h = np.ascontiguousarray(
        np.asarray(w2).astype(bf16).reshape(NM, 128, D).transpose(1, 0, 2))
    in_maps = []
    for c in range(NCORES):
        cs = slice(c * DLOC, (c + 1) * DLOC)
        wq = np.concatenate(
            [w_qkv_eff[:, cs], w_qkv_eff[:, D:][:, cs],
             w_qkv_eff[:, 2 * D:][:, cs]], axis=1)
        wq8 = _fp8(wq, S_W)                      # [1024, 384]
        nws = wq8.astype(np.float32).sum(axis=0) / S_W      # [384]
        wq8p = np.ascontiguousarray(
            wq8.reshape(4, 2, 128, 3 * DLOC).transpose(2, 0, 1, 3))
        rows = shard_rows(c)
        in_maps.append({
            "xt": xt8,
            "xbf": xbf,
            "xsh": np.ascontiguousarray(xf[rows].astype(bf16)),
            "wqkv": wq8p,
            "nws": np.ascontiguousarray(
                nws.reshape(3, 128).T.astype(np.float32)),
            "wout": np.ascontiguousarray(
                np.asarray(w_out)[cs].astype(bf16)),
            "w1": w1h,
            "b1g": b1gh,
            "w2": w2h,
            "masks": masks,
        })
    return in_maps


def kernel(**inputs):
    nc = build()
    in_maps = make_in_maps(**inputs)
    res = bass_utils.run_bass_kernel_spmd(
        nc, in_maps, core_ids=list(range(NCORES)))
    out = np.empty((T, D), np.float32)
    for c in range(NCORES):
        out[shard_rows(c)] = res.results[c]["out"]
    return out.reshape(B, L, D).astype(np.float32)
